# revision 14
# baseline (speedup 1.0000x reference)
"""Multi-head attention (B=2, S=2048, H=32, D=128) on 8 Trainium2 NeuronCores.

Sharding: tensor-parallel across heads (core c owns heads [4c, 4c+4)).
Each core projects q/k/v for all 4096 tokens (weights column-sharded by
head), runs attention for its 4 heads, reshards the context head-major ->
token-major with one AllToAll per head, and runs the full output projection
on its 512-token block, producing o^T [4096, 512] (host transposes).

Scheduling: the tensor engine is the bottleneck (~260 ns per 512-free
matmul regardless of dtype/shape), so the kernel is one continuous PE
stream with the stalls scheduled away:
  P1 k-proj | P2 q-proj | P3 v-proj(b0)  -- dense GEMMs, ScalarE evacuates
  P4 attention(b0): blocks software-pipelined (ctx of block n-1 and its
     softmax tail interleaved into the scores of block n so the PE never
     waits on the Exp activations); v-proj(b1) chains fill remaining gaps
  P5 attention(b1): AllToAll fires per head as it completes; o-proj
     partial-chain bursts for landed groups keep the PE fed
  P6 last two o-proj groups (the second-to-last burst hides the final
     AllToAll) + bf16-accumulator combine + output DMA
PSUM: 2x[128,1024] (scores / kq-proj) + 4x[128,512] (v-proj, ctx, row-sum,
o-proj chains) = exactly 8 banks.
"""

import numpy as np
import ml_dtypes

import concourse.bacc as bacc
import concourse.mybir as mybir
import concourse.tile as tile
from concourse.bass_utils import run_bass_kernel_spmd

bf16 = ml_dtypes.bfloat16

B, S, H, D = 2, 2048, 32, 128
DM = H * D                      # 4096
BT = B * S                      # 4096 tokens total
N_CORES = 8
HL = H // N_CORES               # heads per core = 4
FL = HL * D                     # feature slice per core = 512
TB = BT // N_CORES              # output token block per core = 512
NKT = S // 128                  # 16 k token-tiles per sequence
SCALE = float(D) ** -0.5

F32 = mybir.dt.float32
BF16 = mybir.dt.bfloat16
Exp = mybir.ActivationFunctionType.Exp
Copy = mybir.ActivationFunctionType.Copy

_CACHE = {}


def _build():
    nc = bacc.Bacc("TRN2", target_bir_lowering=False, debug=False,
                   num_devices=N_CORES)

    qT = nc.dram_tensor("qT", [DM, BT], BF16, kind="ExternalInput")
    kT = nc.dram_tensor("kT", [DM, BT], BF16, kind="ExternalInput")
    vT = nc.dram_tensor("vT", [DM, BT], BF16, kind="ExternalInput")
    wqT = nc.dram_tensor("wqT", [DM, FL], BF16, kind="ExternalInput")
    wkT = nc.dram_tensor("wkT", [DM, FL], BF16, kind="ExternalInput")
    wvT = nc.dram_tensor("wvT", [DM, FL], BF16, kind="ExternalInput")
    woT = nc.dram_tensor("woT", [DM, DM], BF16, kind="ExternalInput")
    outT = nc.dram_tensor("outT", [DM, TB], F32, kind="ExternalOutput")

    v_r = vT.ap().rearrange("(kk p) t -> p kk t", p=128)
    wo_r = woT.ap().rearrange("(kk p) f -> p kk f", p=128)
    out_r = outT.ap().rearrange("(fo p) t -> p fo t", p=128)

    with tile.TileContext(nc) as tc:
        with (
            tc.tile_pool(name="persist", bufs=1) as persist,
            tc.tile_pool(name="big", bufs=2, space="PSUM") as bigp,
            tc.tile_pool(name="sm", bufs=4, space="PSUM") as smp,
            tc.tile_pool(name="attn", bufs=1) as attn,
            tc.tile_pool(name="dram", bufs=1, space="DRAM") as dram,
        ):
            qpT = persist.tile([128, HL, BT], BF16, tag="qpT")
            kpT = persist.tile([128, HL, BT], BF16, tag="kpT")
            vp = persist.tile([128, B * NKT, FL], BF16, tag="vp")
            ones_m = persist.tile([128, 128], BF16, tag="ones_m")
            nc.vector.memset(ones_m[:], 1.0)

            in_bufs, out_bufs = [], []
            for h in range(HL):
                in_bufs.append(dram.tile([N_CORES, 128, TB], BF16,
                                         tag=f"ain{h}", name=f"a2a_in{h}"))
                out_bufs.append(dram.tile([N_CORES, 128, TB], BF16,
                                          tag=f"aout{h}", name=f"a2a_out{h}"))

            # ---------------- P1/P2: k then q projection (feature-major) ---
            with (
                tc.tile_pool(name="wkq", bufs=2) as wkq,
                tc.tile_pool(name="xkq", bufs=2) as xkq,
            ):
                for first, (x_dram, w_dram, out_t) in enumerate(
                        ((kT, wkT, kpT), (qT, wqT, qpT))):
                    first = first == 0
                    x_r = x_dram.ap().rearrange("(kk p) t -> p kk t", p=128)
                    w_r = w_dram.ap().rearrange("(kk p) f -> p kk f", p=128)
                    wh = []
                    for kh in range(2):
                        w = wkq.tile([128, 16, FL], BF16, tag="w", name="w")
                        if first and kh == 0:   # small first piece -> fast start
                            nc.sync.dma_start(out=w[:, 0:4, :],
                                              in_=w_r[:, 0:4, :])
                            nc.sync.dma_start(out=w[:, 4:16, :],
                                              in_=w_r[:, 4:16, :])
                        else:
                            nc.sync.dma_start(
                                out=w[:], in_=w_r[:, kh * 16:(kh + 1) * 16, :])
                        wh.append(w)
                    for tch in range(8):       # 512-token chunks
                        pss = [bigp.tile([128, 1024], F32, tag="big",
                                         name=f"pss{mp}") for mp in range(2)]
                        for kh in range(2):    # halves of the contraction
                            xs = xkq.tile([128, 16, 512], BF16, tag="xs")
                            src = x_r[:, kh * 16:(kh + 1) * 16,
                                      tch * 512:(tch + 1) * 512]
                            if first and tch == 0 and kh == 0:
                                nc.sync.dma_start(out=xs[:, 0:4, :],
                                                  in_=src[:, 0:4, :])
                                nc.sync.dma_start(out=xs[:, 4:16, :],
                                                  in_=src[:, 4:16, :])
                            else:
                                nc.sync.dma_start(out=xs[:], in_=src)
                            for ms in range(4):
                                dst = pss[ms // 2][:, (ms % 2) * 512:
                                                   (ms % 2 + 1) * 512]
                                for kk in range(16):
                                    nc.tensor.matmul(
                                        dst,
                                        wh[kh][:, kk,
                                               ms * 128:(ms + 1) * 128],
                                        xs[:, kk, :],
                                        start=(kh == 0 and kk == 0),
                                        stop=(kh == 1 and kk == 15))
                        for mp in range(2):
                            nc.scalar.activation(
                                out_t[:, 2 * mp:2 * mp + 2,
                                      tch * 512:(tch + 1) * 512],
                                pss[mp][:], Copy)

            # ---------------- fill queue (gap-filler steps for the PE) -----
            fill_q = []

            def fill(budget):
                while fill_q and budget > 0:
                    cost, fn = fill_q.pop(0)
                    budget -= cost
                    fn()

            # -------- software-pipelined attention block machinery ---------
            pend = [None]

            def attn_iter(cur):
                """Emit scores+exp for block `cur`; weave in the softmax
                tail and the ctx matmuls of the previous block."""
                p = pend[0]

                def ctx_pair(kt0):
                    if p["ps_c"] is None:
                        p["ps_c"] = smp.tile([128, TB], F32, tag="sm",
                                             name="ps_c")
                    for kt in (kt0, kt0 + 1):
                        nc.tensor.matmul(
                            p["ps_c"][:],
                            vp[:, p["b"] * NKT + kt,
                               p["hl"] * 128:(p["hl"] + 1) * 128],
                            p["pt"][:, kt, :],
                            start=(kt == 0), stop=(kt == NKT - 1))

                def finish_sums():
                    sp = attn.tile([128, TB], BF16, tag="sp", bufs=2)
                    nc.vector.tensor_add(sp[:], p["sp2"][:, 0, :],
                                         p["sp2"][:, 1, :])
                    ps_b = smp.tile([128, TB], F32, tag="sm", name="ps_b")
                    nc.tensor.matmul(ps_b[:], ones_m[:], sp[:],
                                     start=True, stop=True)
                    rsb = attn.tile([128, TB], F32, tag="rsb", bufs=2)
                    nc.vector.reciprocal_approx_fast(rsb[:], ps_b[:])
                    p["rsb"] = rsb

                def finish_ctx():
                    ctxs = attn.tile([128, TB], BF16, tag="ctxs", bufs=2)
                    nc.vector.tensor_tensor(ctxs[:], p["ps_c"][:],
                                            p["rsb"][:],
                                            op=mybir.AluOpType.mult)
                    nc.sync.dma_start(
                        out=in_bufs[p["hl"]][p["b"] * 4 + p["qb"]],
                        in_=ctxs[:])

                if cur is None:            # final flush
                    if p is not None:
                        finish_sums()
                        for g in range(8):
                            ctx_pair(2 * g)
                        finish_ctx()
                        pend[0] = None
                    return

                hl, b, qb = cur
                qs = slice(b * S + qb * TB, b * S + (qb + 1) * TB)
                pt = attn.tile([128, NKT, TB], BF16, tag="pt", bufs=2)
                sp2 = attn.tile([128, 2, TB], BF16, tag="sp2", bufs=2)
                for g in range(8):
                    st = bigp.tile([128, 1024], F32, tag="big")
                    for half in range(2):
                        kt = 2 * g + half
                        nc.tensor.matmul(
                            st[:, half * 512:(half + 1) * 512],
                            kpT[:, hl, b * S + kt * 128:
                                b * S + (kt + 1) * 128],
                            qpT[:, hl, qs],
                            start=True, stop=True)
                    nc.scalar.activation(pt[:, 2 * g:2 * g + 2, :],
                                         st[:], Exp, scale=SCALE)
                    if p is not None and g >= 1:
                        ctx_pair(2 * (g - 1))
                    if g == 7 and p is not None:
                        ctx_pair(14)
                        finish_ctx()       # before add7 so DVE isn't blocked
                    if g == 1:
                        nc.vector.tensor_add(sp2[:], pt[:, 0:2, :],
                                             pt[:, 2:4, :])
                        if p is not None:
                            finish_sums()
                        fill(1)
                    elif g > 1:
                        nc.vector.tensor_add(sp2[:], sp2[:],
                                             pt[:, 2 * g:2 * g + 2, :])
                        if g == 5:
                            fill(1)
                pend[0] = {"hl": hl, "b": b, "qb": qb, "pt": pt, "sp2": sp2,
                           "rsb": None, "ps_c": None}

            # ---------------- P3 + P4 (v-proj scoped) ----------------------
            with (
                tc.tile_pool(name="wvp", bufs=1) as wvp,
                tc.tile_pool(name="xvp", bufs=2) as xvp,
            ):
                wv = wvp.tile([128, 32, FL], BF16, tag="wv")
                wv_r = wvT.ap().rearrange("(kk p) f -> p kk f", p=128)
                nc.sync.dma_start(out=wv[:, 0:16, :], in_=wv_r[:, 0:16, :])
                nc.sync.dma_start(out=wv[:, 16:32, :], in_=wv_r[:, 16:32, :])

                def vproj_chunk_steps(c, evac_scalar):
                    """(cost, fn) steps for one 256-token chunk of v-proj.
                    Each k-tile chain (32 matmuls into one PSUM bank) is
                    split into 4 parts of 8 matmuls for even spreading."""
                    steps = []
                    xh = [None, None]
                    psh = [None]

                    def dma(kh, c=c):
                        xh[kh] = xvp.tile([128, 16, 256], BF16, tag="xs",
                                          name="xs")
                        nc.sync.dma_start(
                            out=xh[kh][:],
                            in_=v_r[:, kh * 16:(kh + 1) * 16,
                                    c * 256:(c + 1) * 256])
                    steps.append((0, lambda: dma(0)))
                    steps.append((0, lambda: dma(1)))
                    for kt2 in range(2):
                        for part in range(4):
                            def pstep(kt2=kt2, part=part, c=c):
                                kh, k8 = part // 2, (part % 2) * 8
                                if part == 0:
                                    psh[0] = smp.tile([128, FL], F32,
                                                      tag="sm", name="ps_v")
                                ps = psh[0]
                                for kk in range(k8, k8 + 8):
                                    nc.tensor.matmul(
                                        ps[:],
                                        xh[kh][:, kk,
                                               kt2 * 128:(kt2 + 1) * 128],
                                        wv[:, kh * 16 + kk, :],
                                        start=(part == 0 and kk == k8),
                                        stop=(part == 3 and kk == k8 + 7))
                                if part == 3:
                                    kt = c * 2 + kt2
                                    if evac_scalar:
                                        nc.scalar.activation(vp[:, kt, :],
                                                             ps[:], Copy)
                                    else:
                                        nc.vector.tensor_copy(vp[:, kt, :],
                                                              ps[:])
                            steps.append((8, pstep))
                    return steps

                # P3: v-proj for batch 0, dense.
                for c in range(8):
                    for _, fn in vproj_chunk_steps(c, evac_scalar=True):
                        fn()

                # P4: attention b0 with v-proj b1 chains as gap fillers.
                for hl in range(HL):
                    fill_q.extend(
                        vproj_chunk_steps(8 + 2 * hl, evac_scalar=False))
                    fill_q.extend(
                        vproj_chunk_steps(9 + 2 * hl, evac_scalar=False))
                    for qb in range(4):
                        attn_iter((hl, 0, qb))
                    fill(10 ** 9)   # drain at head boundary

            # ---------------- P5 + P6 (o-proj scoped) ----------------------
            with tc.tile_pool(name="oproj", bufs=1) as op:
                acc = op.tile([128, 16, 2, TB], BF16, tag="acc")
                ctxg = [None] * HL

                def load_ctxg(h):
                    # shared 2-deep ring: group h reuses the buffer of group
                    # h-2, which the burst schedule has already consumed
                    ctxg[h] = op.tile([128, N_CORES, TB], BF16,
                                      tag="ctxg", bufs=2, name=f"ctxg{h}")
                    nc.sync.dma_start(
                        out=ctxg[h][:],
                        in_=out_bufs[h].rearrange("j p t -> p j t"))

                def collective(h):
                    nc.gpsimd.collective_compute(
                        "AllToAll", mybir.AluOpType.bypass,
                        replica_groups=[list(range(N_CORES))],
                        ins=[in_bufs[h].opt()],
                        outs=[out_bufs[h].opt()])
                    load_ctxg(h)

                def oproj_group(h):
                    """Partial chains for one A2A group: 16 fop x 2 sub.
                    woT rows are host-permuted so group h's 8 contraction
                    tiles are rows [h*1024, (h+1)*1024)."""
                    for fop in range(16):
                        woc = op.tile([128, N_CORES, 256], BF16,
                                      tag="woc", bufs=2, name="woc")
                        nc.sync.dma_start(
                            out=woc[:],
                            in_=wo_r[:, h * N_CORES:(h + 1) * N_CORES,
                                     fop * 256:(fop + 1) * 256])
                        for sub in range(2):
                            ps = smp.tile([128, TB], F32, tag="sm",
                                          name="ps_o")
                            for j in range(N_CORES):
                                nc.tensor.matmul(
                                    ps[:],
                                    woc[:, j, sub * 128:(sub + 1) * 128],
                                    ctxg[h][:, j, :],
                                    start=(j == 0), stop=(j == N_CORES - 1))
                            a = acc[:, fop, sub, :]
                            if h == 0:
                                nc.vector.tensor_copy(a, ps[:])
                            elif h < HL - 1:
                                nc.vector.tensor_add(a, a, ps[:])
                            else:
                                ot = op.tile([128, TB], F32, tag="ot",
                                             bufs=2, name="ot")
                                nc.vector.tensor_add(ot[:], a, ps[:])
                                nc.sync.dma_start(
                                    out=out_r[:, fop * 2 + sub, :],
                                    in_=ot[:])

                # P5: attention b1.  Block (hl,1,3) finishes during the next
                # iteration, so collective(hl) is emitted one block later;
                # the group-0 burst trails its A2A by 7 attention blocks.
                for i in range(16):
                    hl, qb = i // 4, i % 4
                    attn_iter((hl, 1, qb))
                    if i in (4, 8, 12):
                        collective(i // 4 - 1)
                    if i == 11:
                        oproj_group(0)

                # P6: flush the last block, fire the last A2A, then the
                # group-1/2 bursts hide it before group 3 runs.
                attn_iter(None)
                collective(3)
                oproj_group(1)
                oproj_group(2)
                oproj_group(3)

    nc.compile()
    return nc


def _prep_inputs(q, k, v, Wq, Wk, Wv, Wo):
    """Host-side sharding: cast to bf16, transpose to feature-major, slice."""
    q, k, v = (np.asarray(x, dtype=np.float32) for x in (q, k, v))
    Wq, Wk, Wv, Wo = (np.asarray(x, dtype=np.float32)
                      for x in (Wq, Wk, Wv, Wo))
    qT = np.ascontiguousarray(q.reshape(BT, DM).astype(bf16).T)
    kT = np.ascontiguousarray(k.reshape(BT, DM).astype(bf16).T)
    vT = np.ascontiguousarray(v.reshape(BT, DM).astype(bf16).T)
    # woT rows permuted so contraction tile kk' = h*8 + j holds the global
    # feature tile kk = j*HL + h (group-contiguous for the kernel).
    woT = Wo.astype(bf16).T.reshape(N_CORES, HL, 128, DM)
    woT = np.ascontiguousarray(woT.transpose(1, 0, 2, 3).reshape(DM, DM))
    in_maps = []
    for c in range(N_CORES):
        sl = slice(c * FL, (c + 1) * FL)
        in_maps.append({
            "qT": qT, "kT": kT, "vT": vT,
            "wqT": np.ascontiguousarray(Wq[sl, :].astype(bf16).T),
            "wkT": np.ascontiguousarray(Wk[sl, :].astype(bf16).T),
            "wvT": np.ascontiguousarray(Wv[sl, :].astype(bf16).T),
            "woT": woT,
        })
    return in_maps


def run_spmd(inputs, trace=False):
    if "nc" not in _CACHE:
        _CACHE["nc"] = _build()
    nc = _CACHE["nc"]
    in_maps = _prep_inputs(**inputs)
    res = run_bass_kernel_spmd(nc, in_maps, core_ids=list(range(N_CORES)),
                               trace=trace)
    o = np.empty((BT, DM), dtype=np.float32)
    for c in range(N_CORES):
        o[c * TB:(c + 1) * TB, :] = res.results[c]["outT"].T
    return o.reshape(B, S, DM), res


def kernel(q, k, v, Wq, Wk, Wv, Wo):
    o, _ = run_spmd(dict(q=q, k=k, v=v, Wq=Wq, Wk=Wk, Wv=Wv, Wo=Wo))
    return o


# revision 19
# speedup vs baseline: 1.0137x; 1.0137x over previous
"""Multi-head attention (B=2, S=2048, H=32, D=128) on 8 Trainium2 NeuronCores.

Sharding: tensor-parallel across heads (core c owns heads [4c, 4c+4)).
Each core projects q/k/v for all 4096 tokens (weights column-sharded by
head), runs attention for its 4 heads, reshards the context head-major ->
token-major with one AllToAll per head, and runs the full output projection
on its 512-token block, producing o^T [4096, 512] (host transposes).

Scheduling: the tensor engine is the bottleneck (~260 ns per 512-free
matmul regardless of dtype/shape), so the kernel is one continuous PE
stream with the stalls scheduled away:
  P1 k-proj | P2 q-proj | P3 v-proj(b0)  -- dense GEMMs, ScalarE evacuates
  P4 attention(b0): blocks software-pipelined (ctx of block n-1 and its
     softmax tail interleaved into the scores of block n so the PE never
     waits on the Exp activations); v-proj(b1) chains fill remaining gaps
  P5 attention(b1): AllToAll fires per head as it completes; o-proj
     partial-chain bursts for landed groups keep the PE fed
  P6 last two o-proj groups (the second-to-last burst hides the final
     AllToAll) + bf16-accumulator combine + output DMA
PSUM: 2x[128,1024] (scores / kq-proj) + 4x[128,512] (v-proj, ctx, row-sum,
o-proj chains) = exactly 8 banks.
"""

import numpy as np
import ml_dtypes

import concourse.bacc as bacc
import concourse.mybir as mybir
import concourse.tile as tile
from concourse.bass_utils import run_bass_kernel_spmd

bf16 = ml_dtypes.bfloat16

B, S, H, D = 2, 2048, 32, 128
DM = H * D                      # 4096
BT = B * S                      # 4096 tokens total
N_CORES = 8
HL = H // N_CORES               # heads per core = 4
FL = HL * D                     # feature slice per core = 512
TB = BT // N_CORES              # output token block per core = 512
NKT = S // 128                  # 16 k token-tiles per sequence
SCALE = float(D) ** -0.5

F32 = mybir.dt.float32
BF16 = mybir.dt.bfloat16
Exp = mybir.ActivationFunctionType.Exp
Copy = mybir.ActivationFunctionType.Copy

_CACHE = {}


def _build():
    nc = bacc.Bacc("TRN2", target_bir_lowering=False, debug=False,
                   num_devices=N_CORES)

    qT = nc.dram_tensor("qT", [DM, BT], BF16, kind="ExternalInput")
    kT = nc.dram_tensor("kT", [DM, BT], BF16, kind="ExternalInput")
    vT = nc.dram_tensor("vT", [DM, BT], BF16, kind="ExternalInput")
    wqT = nc.dram_tensor("wqT", [DM, FL], BF16, kind="ExternalInput")
    wkT = nc.dram_tensor("wkT", [DM, FL], BF16, kind="ExternalInput")
    wvT = nc.dram_tensor("wvT", [DM, FL], BF16, kind="ExternalInput")
    woT = nc.dram_tensor("woT", [DM, DM], BF16, kind="ExternalInput")
    outT = nc.dram_tensor("outT", [DM, TB], F32, kind="ExternalOutput")

    v_r = vT.ap().rearrange("(kk p) t -> p kk t", p=128)
    wo_r = woT.ap().rearrange("(kk p) f -> p kk f", p=128)
    out_r = outT.ap().rearrange("(fo p) t -> p fo t", p=128)

    with tile.TileContext(nc) as tc:
        with (
            tc.tile_pool(name="persist", bufs=1) as persist,
            tc.tile_pool(name="big", bufs=2, space="PSUM") as bigp,
            tc.tile_pool(name="sm", bufs=4, space="PSUM") as smp,
            tc.tile_pool(name="attn", bufs=1) as attn,
            tc.tile_pool(name="dram", bufs=1, space="DRAM") as dram,
        ):
            qpT = persist.tile([128, HL, BT], BF16, tag="qpT")
            kpT = persist.tile([128, HL, BT], BF16, tag="kpT")
            vp = persist.tile([128, B * NKT, FL], BF16, tag="vp")
            ones_m = persist.tile([128, 128], BF16, tag="ones_m")
            nc.vector.memset(ones_m[:], 1.0)

            in_bufs, out_bufs = [], []
            for h in range(HL):
                in_bufs.append(dram.tile([N_CORES, 128, TB], BF16,
                                         tag=f"ain{h}", name=f"a2a_in{h}"))
                out_bufs.append(dram.tile([N_CORES, 128, TB], BF16,
                                          tag=f"aout{h}", name=f"a2a_out{h}"))

            # Tiny dummy AllToAll issued up front: the first collective pays
            # ~45us of warmup; absorb it here, overlapped with P1 compute.
            warm_in = dram.tile([N_CORES, 16], BF16, tag="warm_in",
                                name="warm_in")
            warm_out = dram.tile([N_CORES, 16], BF16, tag="warm_out",
                                 name="warm_out")
            nc.gpsimd.collective_compute(
                "AllToAll", mybir.AluOpType.bypass,
                replica_groups=[list(range(N_CORES))],
                ins=[warm_in.opt()], outs=[warm_out.opt()])

            # ---------------- P1/P2: k then q projection (feature-major) ---
            with (
                tc.tile_pool(name="wkq", bufs=2) as wkq,
                tc.tile_pool(name="xkq", bufs=2) as xkq,
            ):
                for first, (x_dram, w_dram, out_t) in enumerate(
                        ((kT, wkT, kpT), (qT, wqT, qpT))):
                    first = first == 0
                    x_r = x_dram.ap().rearrange("(kk p) t -> p kk t", p=128)
                    w_r = w_dram.ap().rearrange("(kk p) f -> p kk f", p=128)
                    wh = []
                    for kh in range(2):
                        w = wkq.tile([128, 16, FL], BF16, tag="w", name="w")
                        if first and kh == 0:   # small first piece -> fast start
                            nc.sync.dma_start(out=w[:, 0:4, :],
                                              in_=w_r[:, 0:4, :])
                            nc.sync.dma_start(out=w[:, 4:16, :],
                                              in_=w_r[:, 4:16, :])
                        else:
                            nc.sync.dma_start(
                                out=w[:], in_=w_r[:, kh * 16:(kh + 1) * 16, :])
                        wh.append(w)
                    for tch in range(8):       # 512-token chunks
                        pss = [bigp.tile([128, 1024], F32, tag="big",
                                         name=f"pss{mp}") for mp in range(2)]
                        for kh in range(2):    # halves of the contraction
                            xs = xkq.tile([128, 16, 512], BF16, tag="xs")
                            src = x_r[:, kh * 16:(kh + 1) * 16,
                                      tch * 512:(tch + 1) * 512]
                            if first and tch == 0 and kh == 0:
                                nc.sync.dma_start(out=xs[:, 0:4, :],
                                                  in_=src[:, 0:4, :])
                                nc.sync.dma_start(out=xs[:, 4:16, :],
                                                  in_=src[:, 4:16, :])
                            else:
                                nc.sync.dma_start(out=xs[:], in_=src)
                            for ms in range(4):
                                dst = pss[ms // 2][:, (ms % 2) * 512:
                                                   (ms % 2 + 1) * 512]
                                for kk in range(16):
                                    nc.tensor.matmul(
                                        dst,
                                        wh[kh][:, kk,
                                               ms * 128:(ms + 1) * 128],
                                        xs[:, kk, :],
                                        start=(kh == 0 and kk == 0),
                                        stop=(kh == 1 and kk == 15))
                        for mp in range(2):
                            nc.scalar.activation(
                                out_t[:, 2 * mp:2 * mp + 2,
                                      tch * 512:(tch + 1) * 512],
                                pss[mp][:], Copy)

            # ---------------- fill queue (gap-filler steps for the PE) -----
            fill_q = []

            def fill(budget):
                while fill_q and budget > 0:
                    cost, fn = fill_q.pop(0)
                    budget -= cost
                    fn()

            # -------- software-pipelined attention block machinery ---------
            pend = [None]

            def attn_iter(cur):
                """Emit scores+exp for block `cur`; weave in the softmax
                tail and the ctx matmuls of the previous block."""
                p = pend[0]

                def ctx_pair(kt0):
                    if p["ps_c"] is None:
                        p["ps_c"] = smp.tile([128, TB], F32, tag="sm",
                                             name="ps_c")
                    for kt in (kt0, kt0 + 1):
                        nc.tensor.matmul(
                            p["ps_c"][:],
                            vp[:, p["b"] * NKT + kt,
                               p["hl"] * 128:(p["hl"] + 1) * 128],
                            p["pt"][:, kt, :],
                            start=(kt == 0), stop=(kt == NKT - 1))

                def finish_sums():
                    sp = attn.tile([128, TB], BF16, tag="sp", bufs=2)
                    nc.vector.tensor_add(sp[:], p["sp2"][:, 0, :],
                                         p["sp2"][:, 1, :])
                    ps_b = smp.tile([128, TB], F32, tag="sm", name="ps_b")
                    nc.tensor.matmul(ps_b[:], ones_m[:], sp[:],
                                     start=True, stop=True)
                    rsb = attn.tile([128, TB], F32, tag="rsb", bufs=2)
                    nc.vector.reciprocal_approx_fast(rsb[:], ps_b[:])
                    p["rsb"] = rsb

                def finish_ctx():
                    ctxs = attn.tile([128, TB], BF16, tag="ctxs", bufs=2)
                    nc.vector.tensor_tensor(ctxs[:], p["ps_c"][:],
                                            p["rsb"][:],
                                            op=mybir.AluOpType.mult)
                    nc.sync.dma_start(
                        out=in_bufs[p["hl"]][p["b"] * 4 + p["qb"]],
                        in_=ctxs[:])

                if cur is None:            # final flush
                    if p is not None:
                        finish_sums()
                        for g in range(8):
                            ctx_pair(2 * g)
                        finish_ctx()
                        pend[0] = None
                    return

                hl, b, qb = cur
                qs = slice(b * S + qb * TB, b * S + (qb + 1) * TB)
                pt = attn.tile([128, NKT, TB], BF16, tag="pt", bufs=2)
                sp2 = attn.tile([128, 2, TB], BF16, tag="sp2", bufs=2)
                for g in range(8):
                    st = bigp.tile([128, 1024], F32, tag="big")
                    for half in range(2):
                        kt = 2 * g + half
                        nc.tensor.matmul(
                            st[:, half * 512:(half + 1) * 512],
                            kpT[:, hl, b * S + kt * 128:
                                b * S + (kt + 1) * 128],
                            qpT[:, hl, qs],
                            start=True, stop=True)
                    nc.scalar.activation(pt[:, 2 * g:2 * g + 2, :],
                                         st[:], Exp, scale=SCALE)
                    if p is not None and g >= 1:
                        ctx_pair(2 * (g - 1))
                    if g == 7 and p is not None:
                        ctx_pair(14)
                        finish_ctx()       # before add7 so DVE isn't blocked
                    if g == 1:
                        nc.vector.tensor_add(sp2[:], pt[:, 0:2, :],
                                             pt[:, 2:4, :])
                        if p is not None:
                            finish_sums()
                        fill(1)
                    elif g > 1:
                        nc.vector.tensor_add(sp2[:], sp2[:],
                                             pt[:, 2 * g:2 * g + 2, :])
                        if g == 5:
                            fill(1)
                pend[0] = {"hl": hl, "b": b, "qb": qb, "pt": pt, "sp2": sp2,
                           "rsb": None, "ps_c": None}

            # ---------------- P3 + P4 (v-proj scoped) ----------------------
            with (
                tc.tile_pool(name="wvp", bufs=1) as wvp,
                tc.tile_pool(name="xvp", bufs=2) as xvp,
            ):
                wv = wvp.tile([128, 32, FL], BF16, tag="wv")
                wv_r = wvT.ap().rearrange("(kk p) f -> p kk f", p=128)
                nc.sync.dma_start(out=wv[:, 0:16, :], in_=wv_r[:, 0:16, :])
                nc.sync.dma_start(out=wv[:, 16:32, :], in_=wv_r[:, 16:32, :])

                def vproj_chunk_steps(c, evac_scalar):
                    """(dmas, parts) steps for one 256-token chunk of
                    v-proj.  Each k-tile chain (32 matmuls into one PSUM
                    bank) is split into 4 parts of 8 matmuls."""
                    xh = [None, None]
                    psh = [None]

                    def dma(kh, c=c):
                        xh[kh] = xvp.tile([128, 16, 256], BF16, tag="xs",
                                          name="xs")
                        nc.sync.dma_start(
                            out=xh[kh][:],
                            in_=v_r[:, kh * 16:(kh + 1) * 16,
                                    c * 256:(c + 1) * 256])
                    dmas = [(0, lambda: dma(0)), (0, lambda: dma(1))]
                    parts = []
                    for kt2 in range(2):
                        for part in range(4):
                            def pstep(kt2=kt2, part=part, c=c):
                                kh, k8 = part // 2, (part % 2) * 8
                                if part == 0:
                                    psh[0] = smp.tile([128, FL], F32,
                                                      tag="sm", name="ps_v")
                                ps = psh[0]
                                for kk in range(k8, k8 + 8):
                                    nc.tensor.matmul(
                                        ps[:],
                                        xh[kh][:, kk,
                                               kt2 * 128:(kt2 + 1) * 128],
                                        wv[:, kh * 16 + kk, :],
                                        start=(part == 0 and kk == k8),
                                        stop=(part == 3 and kk == k8 + 7))
                                if part == 3:
                                    kt = c * 2 + kt2
                                    if evac_scalar:
                                        nc.scalar.activation(vp[:, kt, :],
                                                             ps[:], Copy)
                                    else:
                                        nc.vector.tensor_copy(vp[:, kt, :],
                                                              ps[:])
                            parts.append((8, pstep))
                    return dmas, parts

                # Interleave so chunk c+1's DMAs are issued before chunk c's
                # last two parts: the transfer hides under those 16 matmuls.
                chunks = [vproj_chunk_steps(c, evac_scalar=(c < 8))
                          for c in range(16)]
                vseq = list(chunks[0][0])
                for c in range(16):
                    vseq += chunks[c][1][0:6]
                    if c + 1 < 16:
                        vseq += chunks[c + 1][0]
                    vseq += chunks[c][1][6:8]

                # P3: v-proj for batch 0, dense (includes chunk 8's DMAs).
                p3_n = 2 + 8 * 10
                for _, fn in vseq[:p3_n]:
                    fn()

                # P4: attention b0 with v-proj b1 chains as gap fillers.
                rest = vseq[p3_n:]
                per_hl = len(rest) // HL
                for hl in range(HL):
                    end = len(rest) if hl == HL - 1 else (hl + 1) * per_hl
                    fill_q.extend(rest[hl * per_hl:end])
                    for qb in range(4):
                        attn_iter((hl, 0, qb))
                    fill(10 ** 9)   # drain at head boundary

            # ---------------- P5 + P6 (o-proj scoped) ----------------------
            with tc.tile_pool(name="oproj", bufs=1) as op:
                acc = op.tile([128, 16, 2, TB], BF16, tag="acc")
                ctxg = [None] * HL

                def load_ctxg(h):
                    # shared 2-deep ring: group h reuses the buffer of group
                    # h-2, which the burst schedule has already consumed
                    ctxg[h] = op.tile([128, N_CORES, TB], BF16,
                                      tag="ctxg", bufs=2, name=f"ctxg{h}")
                    nc.sync.dma_start(
                        out=ctxg[h][:],
                        in_=out_bufs[h].rearrange("j p t -> p j t"))

                def collective(h, load=True):
                    nc.gpsimd.collective_compute(
                        "AllToAll", mybir.AluOpType.bypass,
                        replica_groups=[list(range(N_CORES))],
                        ins=[in_bufs[h].opt()],
                        outs=[out_bufs[h].opt()])
                    if load:
                        load_ctxg(h)

                def oproj_group(h):
                    """Partial chains for one A2A group: 16 fop x 2 sub.
                    woT rows are host-permuted so group h's 8 contraction
                    tiles are rows [h*1024, (h+1)*1024)."""
                    for fop in range(16):
                        woc = op.tile([128, N_CORES, 256], BF16,
                                      tag="woc", bufs=2, name="woc")
                        nc.sync.dma_start(
                            out=woc[:],
                            in_=wo_r[:, h * N_CORES:(h + 1) * N_CORES,
                                     fop * 256:(fop + 1) * 256])
                        for sub in range(2):
                            ps = smp.tile([128, TB], F32, tag="sm",
                                          name="ps_o")
                            for j in range(N_CORES):
                                nc.tensor.matmul(
                                    ps[:],
                                    woc[:, j, sub * 128:(sub + 1) * 128],
                                    ctxg[h][:, j, :],
                                    start=(j == 0), stop=(j == N_CORES - 1))
                            a = acc[:, fop, sub, :]
                            if h == 0:
                                nc.vector.tensor_copy(a, ps[:])
                            elif h < HL - 1:
                                nc.vector.tensor_add(a, a, ps[:])
                            else:
                                ot = op.tile([128, TB], F32, tag="ot",
                                             bufs=2, name="ot")
                                nc.vector.tensor_add(ot[:], a, ps[:])
                                nc.sync.dma_start(
                                    out=out_r[:, fop * 2 + sub, :],
                                    in_=ot[:])

                # P5: attention b1.  Block (hl,1,3) finishes during the next
                # iteration, so collective(hl) is emitted one block later;
                # the group-0 burst trails its A2A by 7 attention blocks.
                for i in range(16):
                    hl, qb = i // 4, i % 4
                    attn_iter((hl, 1, qb))
                    if i in (4, 8, 12):
                        collective(i // 4 - 1)
                    if i == 11:
                        oproj_group(0)

                # P6: flush the last block, fire the last A2A, then the
                # group-1/2 bursts hide it before group 3 runs.  ctxg3's
                # load is emitted after the g1 burst: its ring slot frees
                # only then, and an earlier emission would block the g1
                # woc weight DMAs behind it in the queue.
                attn_iter(None)
                collective(3, load=False)
                oproj_group(1)
                load_ctxg(3)
                oproj_group(2)
                oproj_group(3)

    nc.compile()
    return nc


def _prep_inputs(q, k, v, Wq, Wk, Wv, Wo):
    """Host-side sharding: cast to bf16, transpose to feature-major, slice."""
    q, k, v = (np.asarray(x, dtype=np.float32) for x in (q, k, v))
    Wq, Wk, Wv, Wo = (np.asarray(x, dtype=np.float32)
                      for x in (Wq, Wk, Wv, Wo))
    qT = np.ascontiguousarray(q.reshape(BT, DM).astype(bf16).T)
    kT = np.ascontiguousarray(k.reshape(BT, DM).astype(bf16).T)
    vT = np.ascontiguousarray(v.reshape(BT, DM).astype(bf16).T)
    # woT rows permuted so contraction tile kk' = h*8 + j holds the global
    # feature tile kk = j*HL + h (group-contiguous for the kernel).
    woT = Wo.astype(bf16).T.reshape(N_CORES, HL, 128, DM)
    woT = np.ascontiguousarray(woT.transpose(1, 0, 2, 3).reshape(DM, DM))
    in_maps = []
    for c in range(N_CORES):
        sl = slice(c * FL, (c + 1) * FL)
        in_maps.append({
            "qT": qT, "kT": kT, "vT": vT,
            "wqT": np.ascontiguousarray(Wq[sl, :].astype(bf16).T),
            "wkT": np.ascontiguousarray(Wk[sl, :].astype(bf16).T),
            "wvT": np.ascontiguousarray(Wv[sl, :].astype(bf16).T),
            "woT": woT,
        })
    return in_maps


def run_spmd(inputs, trace=False):
    if "nc" not in _CACHE:
        _CACHE["nc"] = _build()
    nc = _CACHE["nc"]
    in_maps = _prep_inputs(**inputs)
    res = run_bass_kernel_spmd(nc, in_maps, core_ids=list(range(N_CORES)),
                               trace=trace)
    o = np.empty((BT, DM), dtype=np.float32)
    for c in range(N_CORES):
        o[c * TB:(c + 1) * TB, :] = res.results[c]["outT"].T
    return o.reshape(B, S, DM), res


def kernel(q, k, v, Wq, Wk, Wv, Wo):
    o, _ = run_spmd(dict(q=q, k=k, v=v, Wq=Wq, Wk=Wk, Wv=Wv, Wo=Wo))
    return o
